# revision 8
# baseline (speedup 1.0000x reference)
"""ExpressionAttentionLayer Trainium2 kernel (v2: fp8 DoubleRow PE path).

Math (per reference, algebraically folded):
  fused/q/k projections folded on the host into one [1024,128] weight per
  core; A_bar = softmax(qk)*M / L1 == exp(qk)*M / sum_k(exp(qk)*M) (the
  softmax denominator cancels; logits are tiny so no max-subtraction);
  the key-sum denominator rides as a ones-column appended to V.

Device decomposition: core d = batch d//4, head pair (2*(d%4), 2*(d%4)+1).

v2 changes vs the 124us baseline:
  * qk projection and QK^T (ST) matmuls run fp8e4m3 in DoubleRow perf
    mode (0.5 cycles/row): weights rescaled by exact powers of two
    (Wq*64, Wk*16) so fp8 stays in its normal range; compensated by the
    exp's free scale=2^-10 parameter. q/k are rounded to fp8 on the
    PSUM->SBUF bias-add copy, then remapped to the [32part, 2subK]
    DoubleRow layout via a DRAM bounce (DMA can't remap partitions
    SBUF->SBUF).
  * Masks preloaded into a static [128, 64, 512] SBUF region with 8-tile
    batched DMAs (SP DIRECT2D config cost was ~40us for 64 singles).
  * Output DMAs issued from the gpsimd SWDGE queue (SP relief); a subset
    of mask multiplies runs on gpsimd (DVE relief).
  * AV + epilogue keep bf16 (fp8 there costs ~3% rms output error).
"""

import os
import sys
from collections import defaultdict

for _p in ("/opt/trn_rl_repo", "/root/.axon_site/_ro/trn_rl_repo"):
    if os.path.isdir(_p) and _p not in sys.path:
        sys.path.insert(0, _p)

import numpy as np

import concourse.bass as bass
import concourse.mybir as mybir
import concourse.tile as tile
from concourse import bacc
from concourse.bass_utils import run_bass_kernel_spmd

B, S, D, H, HD = 2, 2048, 512, 8, 64
KX = 2 * D
NCH = KX // 128          # 8 contraction chunks per group
NG = 4                   # token groups (qb granularity for q/k emission)
N_CORES = 8
QB = 512
NQB = S // QB
KT = 128
NKT = S // KT
NT = NQB * NKT
LAG = 4
SCALE = 1.0 / np.sqrt(HD)
QSH, KSH = 64.0, 16.0    # power-of-2 fp8 range rescale for q / k
EXP_SCALE = 1.0 / (QSH * KSH)

f32 = mybir.dt.float32
bf16 = mybir.dt.bfloat16
f8 = mybir.dt.float8e4
DR = mybir.MatmulPerfMode.DoubleRow

M_DT = bf16
P_DT = bf16
X_DT = f8

# tiles whose mask-multiply runs on gpsimd instead of DVE
GP_MOD = int(os.environ.get("KERNEL_GP_MOD", "16"))
GP_OFF = int(os.environ.get("KERNEL_GP_OFF", "3"))
GP_SET = set(T for T in range(NT) if GP_MOD and T % GP_MOD == GP_OFF)

_compiled = None
_last_results = None


def _build():
    nc = bacc.Bacc("TRN2", target_bir_lowering=False, debug=False,
                   num_devices=N_CORES)
    AF = mybir.ActivationFunctionType

    xt = nc.dram_tensor("xt", [128, NG * NCH, QB], f8,
                        kind="ExternalInput").ap()
    # mask pre-tiled on the host: [128 keys-part, T=(qb*16+kt), 512 queries]
    # so each partition's stream for a batch of tiles is contiguous.
    mt = nc.dram_tensor("mt", [128, NT * QB], M_DT, kind="ExternalInput").ap()
    wq = nc.dram_tensor("wq", [128, NCH, 128], f8, kind="ExternalInput").ap()
    wk = nc.dram_tensor("wk", [128, NCH, 128], f8, kind="ExternalInput").ap()
    bq = nc.dram_tensor("bq", [128, 1], f32, kind="ExternalInput").ap()
    bk = nc.dram_tensor("bk", [128, 1], f32, kind="ExternalInput").ap()
    v0 = nc.dram_tensor("v0", [128, NKT, HD + 1], P_DT, kind="ExternalInput").ap()
    v1 = nc.dram_tensor("v1", [128, NKT, HD + 1], P_DT, kind="ExternalInput").ap()
    wo0 = nc.dram_tensor("wo0", [HD, D], bf16, kind="ExternalInput").ap()
    wo1 = nc.dram_tensor("wo1", [HD, D], bf16, kind="ExternalInput").ap()
    out = nc.dram_tensor("out", [S, D], f32, kind="ExternalOutput").ap()

    with tile.TileContext(nc) as tc:
        with tc.tile_pool(name="const", bufs=1) as const, \
             tc.tile_pool(name="ep", bufs=6) as ep, \
             tc.tile_pool(name="pp", bufs=8) as pp, \
             tc.tile_pool(name="sqp", bufs=2) as sqp, \
             tc.tile_pool(name="nsb", bufs=2) as nsb, \
             tc.tile_pool(name="small", bufs=2) as small, \
             tc.tile_pool(name="bcp", bufs=2) as bcp, \
             tc.tile_pool(name="shp", bufs=2) as shp, \
             tc.tile_pool(name="outp", bufs=2) as outp, \
             tc.tile_pool(name="pst", bufs=2, space="PSUM") as pst, \
             tc.tile_pool(name="pacc", bufs=2, space="PSUM") as pacc, \
             tc.tile_pool(name="drp", bufs=2, space="DRAM") as drp:

            # ---- PE warm-up burst + Exp table preload -----------------
            warm_in = const.tile([128, QB], bf16)
            nc.vector.memset(warm_in, 1.0)
            one_f = const.tile([1, 1], f32)
            nc.vector.memset(one_f, 1.0)
            warm_o = const.tile([1, 8], f32)
            for i in range(13):
                warm_ps = pacc.tile([128, QB], f32, tag="qk",
                                    name=f"warm{i}")
                nc.tensor.matmul(warm_ps, warm_in[:, 0:128], warm_in,
                                 start=True, stop=True)
            nc.scalar.activation(warm_o, warm_in[0:1, 0:8], AF.Exp)

            # ---- static input regions ---------------------------------
            xt_s = const.tile([128, NG * NCH, QB], f8)
            msk_s = const.tile([128, NT, QB], M_DT)
            wq_s = const.tile([128, NCH, 128], f8)
            wk_s = const.tile([128, NCH, 128], f8)
            bq_s = const.tile([128, 1], f32)
            bk_s = const.tile([128, 1], f32)
            v0_s = const.tile([128, NKT, HD + 1], P_DT)
            v1_s = const.tile([128, NKT, HD + 1], P_DT)
            wo0_s = const.tile([HD, D], bf16)
            wo1_s = const.tile([HD, D], bf16)

            def issue_xt(g):
                nc.sync.dma_start(out=xt_s[:, g * NCH:(g + 1) * NCH, :],
                                  in_=xt[:, g * NCH:(g + 1) * NCH, :])

            def issue_mask_batch(start, count):
                nc.sync.dma_start(
                    out=msk_s[:, start:start + count, :],
                    in_=mt[:, start * QB:(start + count) * QB])

            # SWDGE warm-up: a tiny dummy DMA so the software queue's
            # startup latency is paid before the first real output.
            swdge_warm = drp.tile([1, 8], f32, tag="swarm", name="swarm")
            nc.gpsimd.dma_start(out=swdge_warm, in_=warm_o)

            # sync-queue order tuned so early-needed data isn't stuck
            # behind bulk transfers (the HWDGE queue is FIFO).
            issue_xt(0)
            nc.sync.dma_start(out=wq_s, in_=wq)
            nc.sync.dma_start(out=wk_s, in_=wk)
            nc.sync.dma_start(out=bq_s, in_=bq)
            nc.sync.dma_start(out=bk_s, in_=bk)
            issue_mask_batch(0, 8)
            issue_xt(1)
            issue_xt(2)
            nc.sync.dma_start(out=v0_s, in_=v0)
            nc.sync.dma_start(out=v1_s, in_=v1)
            issue_mask_batch(8, 4)
            issue_xt(3)
            issue_mask_batch(12, 4)
            nc.sync.dma_start(out=wo0_s, in_=wo0)
            nc.sync.dma_start(out=wo1_s, in_=wo1)

            # q^T / k^T in DoubleRow layout: [64 part, 2 subK, 512 tok] fp8,
            # head h on partitions 32h..32h+31; free block j holds head-dims
            # {64h + 32j + p}.
            qT_g = [const.tile([64, 2, QB], f8, name=f"qT{g}")
                    for g in range(NG)]
            kT_g = [const.tile([64, 2, QB], f8, name=f"kT{g}")
                    for g in range(NG)]

            group_state = {}

            def emit_group_part(g, t):
                """t=0: q chain; t=1: k chain + DRAM-bounce remap (on the
                gpsimd SWDGE queue so it isn't FIFO-blocked behind the
                sync queue's bulk mask/xt stream)."""
                if t == 0:
                    group_state[g] = sqp.tile([128, 2, QB], f8, tag="sqk",
                                              name=f"sqk{g}")
                s_qk = group_state[g]
                w_s, b_s = (wq_s, bq_s) if t == 0 else (wk_s, bk_s)
                acc = pacc.tile([128, QB], f32, tag="qk", name=f"qk{t}_{g}")
                for i in range(NCH // 2):
                    nc.tensor.matmul(
                        acc, w_s[:, 2 * i:2 * i + 2, :],
                        xt_s[:, g * NCH + 2 * i:g * NCH + 2 * i + 2, :],
                        start=(i == 0), stop=(i == NCH // 2 - 1),
                        perf_mode=DR)
                nc.vector.tensor_scalar_add(s_qk[:, t, :], acc, b_s)
                if t == 1:
                    qkd = drp.tile([128, 2 * QB], f8, tag="qkd",
                                   name=f"qkd{g}")
                    nc.gpsimd.dma_start(out=qkd, in_=s_qk)
                    # DMA APs are limited to 3 dims: one remap DMA per
                    # (q/k, head) with dims (p32, subK j, token c).
                    for u, dstT in ((0, qT_g[g]), (1, kT_g[g])):
                        for h in range(2):
                            src = bass.AP(
                                tensor=qkd.tensor,
                                offset=qkd.offset + h * 128 * QB + u * QB,
                                ap=[[2 * QB, 32],    # p32
                                    [64 * QB, 2],    # j: sub-K block
                                    [1, QB]])        # c: token
                            nc.gpsimd.dma_start(
                                out=dstT[32 * h:32 * h + 32, :, :],
                                in_=src)

            def emit_group(g):
                emit_group_part(g, 0)
                emit_group_part(g, 1)

            # ---- pipeline body helpers --------------------------------
            p_tiles = {}
            num = {}

            def emit_st(T):
                qb, kt = divmod(T, NKT)
                g, c = divmod(kt, NG)
                st = pst.tile([128, 2 * QB], f32, tag="st", name="st")
                for h in range(2):
                    nc.tensor.matmul(
                        st[:, h * QB:(h + 1) * QB],
                        kT_g[g][32 * h:32 * h + 32, :, c * KT:(c + 1) * KT],
                        qT_g[qb][32 * h:32 * h + 32, :, :],
                        start=True, stop=True, perf_mode=DR)
                e_t = ep.tile([128, 2 * QB], P_DT, tag="e", name="e_t")
                nc.scalar.activation(e_t, st, AF.Exp, scale=EXP_SCALE)
                p_t = pp.tile([128, 2 * QB], P_DT, tag="p", name="p_t")
                m0 = msk_s.offset + T * QB
                if T in GP_SET:
                    m = bass.AP(tensor=msk_s.tensor, offset=m0,
                                ap=[list(msk_s.ap[0]), [1, QB]])
                    for h in range(2):
                        nc.gpsimd.tensor_mul(
                            p_t[:, h * QB:(h + 1) * QB],
                            e_t[:, h * QB:(h + 1) * QB], m)
                else:
                    mb = bass.AP(tensor=msk_s.tensor, offset=m0,
                                 ap=[list(msk_s.ap[0]), [0, 2], [1, QB]])
                    e3 = bass.AP(tensor=e_t.tensor, offset=e_t.offset,
                                 ap=[list(e_t.ap[0]), [QB, 2], [1, QB]])
                    p3 = bass.AP(tensor=p_t.tensor, offset=p_t.offset,
                                 ap=[list(p_t.ap[0]), [QB, 2], [1, QB]])
                    nc.vector.tensor_mul(p3, e3, mb)
                p_tiles[T] = p_t

            def emit_av(T):
                qb, kt = divmod(T, NKT)
                if kt == 0:
                    num[qb] = [pacc.tile([HD + 1, QB], f32, tag="num",
                                         name=f"num{h}_{qb}")
                               for h in range(2)]
                p_t = p_tiles.pop(T)
                for h, v_s in ((0, v0_s), (1, v1_s)):
                    nc.tensor.matmul(num[qb][h], v_s[:, kt, :],
                                     p_t[:, h * QB:(h + 1) * QB],
                                     start=(kt == 0), stop=(kt == NKT - 1))

            def stage1(qb):
                """At qb's last AV: drain numerators out of PSUM, start the
                1/den bounce. Returns state for the deferred stages."""
                n0, n1 = num.pop(qb)
                st8 = {}
                for h, n in ((0, n0), (1, n1)):
                    den = small.tile([1, QB], f32, tag=f"den{h}", name="den")
                    nc.vector.tensor_copy(den, n[HD:HD + 1, :])
                    ns = nsb.tile([HD, QB], f32, tag=f"nsb{h}",
                                  name=f"nsb{h}")
                    nc.vector.tensor_copy(ns, n[0:HD, :])
                    rec = small.tile([1, QB], f32, tag=f"rec{h}", name="rec")
                    nc.vector.reciprocal_approx_fast(rec, den)
                    rec_d = drp.tile([1, QB], f32, tag=f"recd{h}",
                                     name="rec_d")
                    nc.sync.dma_start(out=rec_d, in_=rec)
                    bc = bcp.tile([HD, QB], f32, tag=f"bc{h}", name="bc")
                    rb = bass.AP(tensor=rec_d.tensor, offset=rec_d.offset,
                                 ap=[[0, HD], [1, QB]])
                    nc.sync.dma_start(out=bc, in_=rb)
                    st8[h] = (ns, bc)
                return st8

            def make_sh(st8, h, shs):
                def cl():
                    ns, bc = st8[h]
                    sh = shp.tile([HD, QB], bf16, tag=f"sh{h}", name=f"sh{h}")
                    nc.vector.tensor_mul(sh, ns, bc)
                    shs[h] = sh
                return cl

            def make_proj(qb, shs, blk):
                def cl():
                    pr = pacc.tile([128, D], f32, tag="qk", name="pr")
                    nc.tensor.matmul(pr, shs[0][:, blk * 128:(blk + 1) * 128],
                                     wo0_s, start=True, stop=False)
                    nc.tensor.matmul(pr, shs[1][:, blk * 128:(blk + 1) * 128],
                                     wo1_s, start=False, stop=True)
                    rows = slice(qb * QB + blk * 128,
                                 qb * QB + (blk + 1) * 128)
                    o_t = outp.tile([128, D], f32, tag="o", name="o_t")
                    nc.vector.tensor_copy(o_t, pr)
                    nc.gpsimd.dma_start(out=out[rows, :], in_=o_t)
                return cl

            # ---- the flat pipeline ------------------------------------
            emit_group(0)
            emit_group(1)

            schedule = defaultdict(list)
            for T in range(NT + LAG):
                if T < NT:
                    if T in (4, 5):
                        emit_group_part(2, T - 4)
                    elif T in (8, 9):
                        emit_group_part(3, T - 8)
                    if T in (2, 10, 18, 26, 34, 42):
                        issue_mask_batch(16 + (T - 2), 8)
                    emit_st(T)
                for cl in schedule.pop(T, []):
                    cl()
                if T >= LAG:
                    TT = T - LAG
                    emit_av(TT)
                    qb2, kt2 = divmod(TT, NKT)
                    if kt2 == NKT - 1 and qb2 < NQB - 1:
                        st8 = stage1(qb2)
                        shs = {}
                        schedule[T + 2].append(make_sh(st8, 0, shs))
                        schedule[T + 3].append(make_sh(st8, 1, shs))
                        for b in range(4):
                            schedule[T + 5 + 2 * b].append(
                                make_proj(qb2, shs, b))

            # ---- tail: last qb, latency-optimized ---------------------
            # Unscaled per-head projections start as soon as the numerators
            # are copied out (keeps the PE warm); 1/den is transposed into
            # per-partition layout with 8 rank-1 PE matmuls (no DRAM bounce)
            # and applied as a per-partition scale, split across ACT + DVE.
            qb = NQB - 1
            n0, n1 = num.pop(qb)
            dd = small.tile([1, 2 * QB], f32, tag="dd", name="dd")
            nc.vector.tensor_copy(dd[:, 0:QB], n0[HD:HD + 1, :])
            nc.vector.tensor_copy(dd[:, QB:2 * QB], n1[HD:HD + 1, :])
            rec = small.tile([1, 2 * QB], f32, tag="rec2", name="rec")
            nc.vector.reciprocal_approx_fast(rec, dd)
            nss = []
            for h, n in ((0, n0), (1, n1)):
                ns = nsb.tile([HD, QB], bf16, tag=f"nst{h}", name=f"nst{h}")
                nc.vector.tensor_copy(ns, n[0:HD, :])
                nss.append(ns)
            prs = []
            for b in (0, 1):
                st_blk = pst.tile([128, 2 * QB], f32, tag="st", name="st_pr")
                prs.append((st_blk[:, 0:QB], st_blk[:, QB:2 * QB]))
            q0 = pacc.tile([128, D], f32, tag="qk", name="prq0")
            q1 = pacc.tile([128, D], f32, tag="qk", name="prq1")
            prs.append((q0, q1))

            def proj_mm(b):
                for h, ns, wo_s in ((0, nss[0], wo0_s), (1, nss[1], wo1_s)):
                    nc.tensor.matmul(prs[b][h],
                                     ns[:, b * 128:(b + 1) * 128],
                                     wo_s, start=True, stop=True)

            def scale_blk(b):
                t0 = small.tile([128, D], f32, tag="t0", name="t0")
                nc.scalar.mul(t0, prs[b][0], rec_t[:, b:b + 1])
                o_t = outp.tile([128, D], f32, tag="o", name="o_t")
                nc.vector.scalar_tensor_tensor(
                    o_t, prs[b][1], rec_t[:, 4 + b:4 + b + 1], t0,
                    mybir.AluOpType.mult, mybir.AluOpType.add)
                rows = slice(qb * QB + b * 128, qb * QB + (b + 1) * 128)
                nc.gpsimd.dma_start(out=out[rows, :], in_=o_t)

            proj_mm(0)
            proj_mm(1)
            proj_mm(2)
            rec_tp = pacc.tile([128, 8], f32, tag="num", name="rec_tp")
            for c in range(8):
                nc.tensor.matmul(rec_tp[:, c:c + 1],
                                 rec[:, c * 128:(c + 1) * 128], one_f,
                                 start=True, stop=True)
            rec_t = small.tile([128, 8], f32, tag="rect", name="rec_t")
            nc.vector.tensor_copy(rec_t, rec_tp)
            scale_blk(0)
            st_blk = pst.tile([128, 2 * QB], f32, tag="st", name="st_pr")
            prs.append((st_blk[:, 0:QB], st_blk[:, QB:2 * QB]))
            proj_mm(3)
            scale_blk(1)
            scale_blk(2)
            scale_blk(3)

    nc.compile()
    return nc


def _get_compiled():
    global _compiled
    if _compiled is None:
        _compiled = _build()
    return _compiled


def kernel(gene_emb, expr_emb, V, M, fused_W, fused_b, Wq, bq, Wk, bk,
           out_W, out_b):
    gene_emb = np.asarray(gene_emb, dtype=np.float32)
    expr_emb = np.asarray(expr_emb, dtype=np.float32)
    V = np.asarray(V, dtype=np.float32)
    M = np.asarray(M, dtype=np.float32)
    fused_W = np.asarray(fused_W, dtype=np.float32)
    fused_b = np.asarray(fused_b, dtype=np.float32)
    Wq_ = np.asarray(Wq, dtype=np.float32)
    bq_ = np.asarray(bq, dtype=np.float32)
    Wk_ = np.asarray(Wk, dtype=np.float32)
    bk_ = np.asarray(bk, dtype=np.float32)
    out_W = np.asarray(out_W, dtype=np.float32)
    out_b = np.asarray(out_b, dtype=np.float32)

    nc = _get_compiled()

    import ml_dtypes
    m_np = ml_dtypes.bfloat16
    p_np = ml_dtypes.bfloat16
    f8_np = ml_dtypes.float8_e4m3

    def to_f8(a):
        return np.clip(a, -240.0, 240.0).astype(f8_np)

    fW = fused_W.astype(np.float64)
    Wqc = (fW @ Wq_.astype(np.float64)) * SCALE * QSH
    bqc = ((fused_b.astype(np.float64) @ Wq_.astype(np.float64) + bq_)
           * SCALE * QSH)
    Wkc = (fW @ Wk_.astype(np.float64)) * KSH
    bkc = (fused_b.astype(np.float64) @ Wk_.astype(np.float64) + bk_) * KSH

    def chunk_major(a, nch):  # [nch*128, F] -> [128, nch, F]
        F = a.shape[1]
        return np.ascontiguousarray(
            a.reshape(nch, 128, F).transpose(1, 0, 2))

    xt_b, mt_b = [], []
    for b in range(B):
        XT = np.concatenate([gene_emb[b], expr_emb[b]], axis=-1).T  # [1024,S]
        xg = XT.reshape(NCH, 128, NG, QB).transpose(1, 2, 0, 3)
        xt_b.append(to_f8(np.ascontiguousarray(
            xg.reshape(128, NG * NCH, QB))))
        # mask layout [128 keys-part, (qb,kt)=T, 512 queries] so one
        # partition's stream over a run of tiles is contiguous in DRAM.
        mtt = M[b].T.reshape(NKT, KT, NQB, QB).transpose(1, 2, 0, 3)
        mt_b.append(np.ascontiguousarray(
            mtt.reshape(KT, NT * QB)).astype(m_np))

    ones_col = np.ones((S, 1), np.float32)
    in_maps = []
    for d in range(N_CORES):
        b, p = d // 4, d % 4
        h0 = 2 * p
        cols = slice(p * 128, (p + 1) * 128)
        vs = []
        for h in (h0, h0 + 1):
            Vh = np.concatenate([V[b, :, h, :], ones_col], axis=1)  # [S,65]
            vs.append(chunk_major(Vh, NKT).astype(p_np))
        in_maps.append({
            "xt": xt_b[b],
            "mt": mt_b[b],
            "wq": to_f8(chunk_major(Wqc[:, cols].astype(np.float32), NCH)),
            "wk": to_f8(chunk_major(Wkc[:, cols].astype(np.float32), NCH)),
            "bq": bqc[cols].astype(np.float32).reshape(128, 1),
            "bk": bkc[cols].astype(np.float32).reshape(128, 1),
            "v0": vs[0],
            "v1": vs[1],
            "wo0": np.ascontiguousarray(
                out_W[h0 * HD:(h0 + 1) * HD, :]).astype(ml_dtypes.bfloat16),
            "wo1": np.ascontiguousarray(
                out_W[(h0 + 1) * HD:(h0 + 2) * HD, :]).astype(
                    ml_dtypes.bfloat16),
        })

    global _last_results
    n_run = int(os.environ.get("KERNEL_CORES", N_CORES))
    if n_run < N_CORES:
        in_maps = in_maps[:1] * N_CORES
    res = run_bass_kernel_spmd(nc, in_maps[:n_run],
                               core_ids=list(range(n_run)))
    if n_run < N_CORES:
        res.results = list(res.results) * (N_CORES // n_run)
    _last_results = res

    final = np.broadcast_to(out_b, (B, S, D)).astype(np.float32).copy()
    for d in range(N_CORES):
        final[d // 4] += res.results[d]["out"]
    return final


# revision 9
# speedup vs baseline: 1.1626x; 1.1626x over previous
"""ExpressionAttentionLayer Trainium2 kernel (v3).

Math (per reference, algebraically folded):
  fused/q/k projections folded on the host into one [1024,128] weight per
  core; A_bar = softmax(qk)*M / L1 == exp(qk)*M / sum_k(exp(qk)*M) (the
  softmax denominator cancels; logits are tiny so no max-subtraction);
  the key-sum denominator rides as a ones-column appended to V.

Device decomposition: core d = batch d//4, head pair (2*(d%4), 2*(d%4)+1).

v3 vs the 124us baseline:
  * qk projection runs fp8e4m3 DoubleRow (4 chunk-pair matmuls instead of
    8): weights rescaled by exact powers of two (x64/x16) to keep fp8 in
    its normal range; undone exactly in the PSUM->SBUF bias-add via
    tensor_scalar (acc * 2^-6|-4 + bias). q/k/ST stay bf16 — DoubleRow on
    ST lost fast-weight-load and was a net loss (265ns vs 206ns/MM).
  * V padded from 65 to 128 columns with zeros: NumWeights==128 enables
    FWL on the AV matmuls (LDWEIGHTS was serializing ~100ns per AV MM).
  * Masks preloaded into a static [128, 64, 512] SBUF region with batched
    DMAs (8-tile batches; SP DIRECT2D config time was ~40us for singles).
  * Output DMAs on the gpsimd SWDGE queue; 1/4 of mask multiplies on
    gpsimd (DVE relief; DVE was within ~5us of the ACT exp floor).
"""

import os
import sys
from collections import defaultdict

for _p in ("/opt/trn_rl_repo", "/root/.axon_site/_ro/trn_rl_repo"):
    if os.path.isdir(_p) and _p not in sys.path:
        sys.path.insert(0, _p)

import numpy as np

import concourse.bass as bass
import concourse.mybir as mybir
import concourse.tile as tile
from concourse import bacc
from concourse.bass_utils import run_bass_kernel_spmd

B, S, D, H, HD = 2, 2048, 512, 8, 64
KX = 2 * D
NCH = KX // 128          # 8 contraction chunks per group
NG = 4                   # token groups
N_CORES = 8
QB = 512
NQB = S // QB
KT = 128
NKT = S // KT
NT = NQB * NKT
LAG = 4
VP = 128                 # V padded to 128 cols (ones at 64, zeros above)
SCALE = 1.0 / np.sqrt(HD)
QSH, KSH = 64.0, 16.0    # power-of-2 fp8 range rescale for q / k weights
NWARM = int(os.environ.get("KERNEL_NWARM", "6"))

f32 = mybir.dt.float32
bf16 = mybir.dt.bfloat16
f8 = mybir.dt.float8e4
DR = mybir.MatmulPerfMode.DoubleRow

M_DT = bf16
P_DT = bf16
X_DT = f8

# tiles whose mask-multiply runs on gpsimd instead of DVE
GP_MOD = int(os.environ.get("KERNEL_GP_MOD", "4"))
GP_OFF = int(os.environ.get("KERNEL_GP_OFF", "2"))
GP_SET = set(T for T in range(NT) if GP_MOD and T % GP_MOD == GP_OFF)

_compiled = None
_last_results = None


def _build():
    nc = bacc.Bacc("TRN2", target_bir_lowering=False, debug=False,
                   num_devices=N_CORES)
    AF = mybir.ActivationFunctionType

    xt = nc.dram_tensor("xt", [128, NG * NCH, QB], X_DT,
                        kind="ExternalInput").ap()
    # mask pre-tiled on the host: [128 keys-part, T=(qb*16+kt) * 512 q] so
    # each partition's stream over a run of tiles is contiguous in DRAM.
    mt = nc.dram_tensor("mt", [128, NT * QB], M_DT, kind="ExternalInput").ap()
    wq = nc.dram_tensor("wq", [128, NCH, 128], X_DT, kind="ExternalInput").ap()
    wk = nc.dram_tensor("wk", [128, NCH, 128], X_DT, kind="ExternalInput").ap()
    bq = nc.dram_tensor("bq", [128, 1], f32, kind="ExternalInput").ap()
    bk = nc.dram_tensor("bk", [128, 1], f32, kind="ExternalInput").ap()
    v0 = nc.dram_tensor("v0", [128, NKT, VP], P_DT, kind="ExternalInput").ap()
    v1 = nc.dram_tensor("v1", [128, NKT, VP], P_DT, kind="ExternalInput").ap()
    wo0 = nc.dram_tensor("wo0", [HD, D], bf16, kind="ExternalInput").ap()
    wo1 = nc.dram_tensor("wo1", [HD, D], bf16, kind="ExternalInput").ap()
    out = nc.dram_tensor("out", [S, D], f32, kind="ExternalOutput").ap()

    with tile.TileContext(nc) as tc:
        with tc.tile_pool(name="const", bufs=1) as const, \
             tc.tile_pool(name="ep", bufs=6) as ep, \
             tc.tile_pool(name="pp", bufs=8) as pp, \
             tc.tile_pool(name="nsb", bufs=2) as nsb, \
             tc.tile_pool(name="small", bufs=2) as small, \
             tc.tile_pool(name="bcp", bufs=2) as bcp, \
             tc.tile_pool(name="shp", bufs=2) as shp, \
             tc.tile_pool(name="outp", bufs=2) as outp, \
             tc.tile_pool(name="pst", bufs=2, space="PSUM") as pst, \
             tc.tile_pool(name="pacc", bufs=2, space="PSUM") as pacc, \
             tc.tile_pool(name="drp", bufs=2, space="DRAM") as drp:

            # ---- PE warm-up burst + Exp table preload -----------------
            warm_in = const.tile([128, QB], bf16)
            nc.vector.memset(warm_in, 1.0)
            one_f = const.tile([1, 1], f32)
            nc.vector.memset(one_f, 1.0)
            warm_o = const.tile([1, 8], f32)
            for i in range(NWARM):
                warm_ps = pacc.tile([128, QB], f32, tag="qk",
                                    name=f"warm{i}")
                nc.tensor.matmul(warm_ps, warm_in[:, 0:128], warm_in,
                                 start=True, stop=True)
            nc.scalar.activation(warm_o, warm_in[0:1, 0:8], AF.Exp)

            # ---- static input regions ---------------------------------
            xt_s = const.tile([128, NG * NCH, QB], X_DT)
            msk_s = const.tile([128, NT, QB], M_DT)
            wq_s = const.tile([128, NCH, 128], X_DT)
            wk_s = const.tile([128, NCH, 128], X_DT)
            bq_s = const.tile([128, 1], f32)
            bk_s = const.tile([128, 1], f32)
            v0_s = const.tile([128, NKT, VP], P_DT)
            v1_s = const.tile([128, NKT, VP], P_DT)
            wo0_s = const.tile([HD, D], bf16)
            wo1_s = const.tile([HD, D], bf16)

            def issue_xt(g):
                nc.sync.dma_start(out=xt_s[:, g * NCH:(g + 1) * NCH, :],
                                  in_=xt[:, g * NCH:(g + 1) * NCH, :])

            def issue_mask_batch(start, count):
                nc.sync.dma_start(
                    out=msk_s[:, start:start + count, :],
                    in_=mt[:, start * QB:(start + count) * QB])

            # SWDGE warm-up: a tiny dummy DMA so the software queue's
            # startup latency is paid before the first real output.
            swdge_warm = drp.tile([1, 8], f32, tag="swarm", name="swarm")
            nc.gpsimd.dma_start(out=swdge_warm, in_=warm_o)

            # sync-queue order tuned so early-needed data isn't stuck
            # behind bulk transfers (the HWDGE queue is FIFO).
            issue_xt(0)
            nc.sync.dma_start(out=wq_s, in_=wq)
            nc.sync.dma_start(out=wk_s, in_=wk)
            nc.sync.dma_start(out=bq_s, in_=bq)
            nc.sync.dma_start(out=bk_s, in_=bk)
            issue_mask_batch(0, 8)
            issue_xt(1)
            nc.sync.dma_start(out=v0_s, in_=v0)
            nc.sync.dma_start(out=v1_s, in_=v1)
            issue_xt(2)
            issue_mask_batch(8, 4)
            issue_xt(3)
            issue_mask_batch(12, 4)
            nc.sync.dma_start(out=wo0_s, in_=wo0)
            nc.sync.dma_start(out=wo1_s, in_=wo1)

            qT_g = [const.tile([128, QB], bf16, name=f"qT{g}")
                    for g in range(NG)]
            kT_g = [const.tile([128, QB], bf16, name=f"kT{g}")
                    for g in range(NG)]

            def emit_group_part(g, t):
                """One DoubleRow fp8 projection chain (t=0: q, t=1: k);
                the power-of-2 weight rescale is undone on the PSUM->SBUF
                copy: dst = acc * (1/QSH|1/KSH) + bias."""
                w_s, b_s, sh, dst = (
                    (wq_s, bq_s, 1.0 / QSH, qT_g[g]) if t == 0
                    else (wk_s, bk_s, 1.0 / KSH, kT_g[g]))
                acc = pacc.tile([128, QB], f32, tag="qk", name=f"qk{t}_{g}")
                for i in range(NCH // 2):
                    nc.tensor.matmul(
                        acc, w_s[:, 2 * i:2 * i + 2, :],
                        xt_s[:, g * NCH + 2 * i:g * NCH + 2 * i + 2, :],
                        start=(i == 0), stop=(i == NCH // 2 - 1),
                        perf_mode=DR)
                nc.vector.tensor_scalar(dst, acc, sh, b_s,
                                        mybir.AluOpType.mult,
                                        mybir.AluOpType.add)

            def emit_group(g):
                emit_group_part(g, 0)
                emit_group_part(g, 1)

            # ---- pipeline body helpers --------------------------------
            p_tiles = {}
            num = {}

            def emit_st(T):
                qb, kt = divmod(T, NKT)
                g, c = divmod(kt, NG)
                st = pst.tile([128, 2 * QB], f32, tag="st", name="st")
                for h in range(2):
                    nc.tensor.matmul(
                        st[:, h * QB:(h + 1) * QB],
                        kT_g[g][h * HD:(h + 1) * HD, c * KT:(c + 1) * KT],
                        qT_g[qb][h * HD:(h + 1) * HD, :],
                        start=True, stop=True,
                        tile_position=(h * HD, 0))
                e_t = ep.tile([128, 2 * QB], P_DT, tag="e", name="e_t")
                nc.scalar.activation(e_t, st, AF.Exp)
                p_t = pp.tile([128, 2 * QB], P_DT, tag="p", name="p_t")
                m0 = msk_s.offset + T * QB
                if T in GP_SET:
                    m = bass.AP(tensor=msk_s.tensor, offset=m0,
                                ap=[list(msk_s.ap[0]), [1, QB]])
                    for h in range(2):
                        nc.gpsimd.tensor_mul(
                            p_t[:, h * QB:(h + 1) * QB],
                            e_t[:, h * QB:(h + 1) * QB], m)
                else:
                    mb = bass.AP(tensor=msk_s.tensor, offset=m0,
                                 ap=[list(msk_s.ap[0]), [0, 2], [1, QB]])
                    e3 = bass.AP(tensor=e_t.tensor, offset=e_t.offset,
                                 ap=[list(e_t.ap[0]), [QB, 2], [1, QB]])
                    p3 = bass.AP(tensor=p_t.tensor, offset=p_t.offset,
                                 ap=[list(p_t.ap[0]), [QB, 2], [1, QB]])
                    nc.vector.tensor_mul(p3, e3, mb)
                p_tiles[T] = p_t

            def emit_av(T):
                qb, kt = divmod(T, NKT)
                if kt == 0:
                    num[qb] = [pacc.tile([VP, QB], f32, tag="num",
                                         name=f"num{h}_{qb}")
                               for h in range(2)]
                p_t = p_tiles.pop(T)
                for h, v_s in ((0, v0_s), (1, v1_s)):
                    nc.tensor.matmul(num[qb][h], v_s[:, kt, :],
                                     p_t[:, h * QB:(h + 1) * QB],
                                     start=(kt == 0), stop=(kt == NKT - 1))

            def stage1(qb):
                """At qb's last AV: drain numerators out of PSUM, start the
                1/den bounce. Returns state for the deferred stages."""
                n0, n1 = num.pop(qb)
                st8 = {}
                for h, n in ((0, n0), (1, n1)):
                    den = small.tile([1, QB], f32, tag=f"den{h}", name="den")
                    nc.vector.tensor_copy(den, n[HD:HD + 1, :])
                    ns = nsb.tile([HD, QB], f32, tag=f"nsb{h}",
                                  name=f"nsb{h}")
                    nc.vector.tensor_copy(ns, n[0:HD, :])
                    rec = small.tile([1, QB], f32, tag=f"rec{h}", name="rec")
                    nc.vector.reciprocal_approx_fast(rec, den)
                    rec_d = drp.tile([1, QB], f32, tag=f"recd{h}",
                                     name="rec_d")
                    nc.sync.dma_start(out=rec_d, in_=rec)
                    bc = bcp.tile([HD, QB], f32, tag=f"bc{h}", name="bc")
                    rb = bass.AP(tensor=rec_d.tensor, offset=rec_d.offset,
                                 ap=[[0, HD], [1, QB]])
                    nc.sync.dma_start(out=bc, in_=rb)
                    st8[h] = (ns, bc)
                return st8

            def make_sh(st8, h, shs):
                def cl():
                    ns, bc = st8[h]
                    sh = shp.tile([HD, QB], bf16, tag=f"sh{h}", name=f"sh{h}")
                    nc.vector.tensor_mul(sh, ns, bc)
                    shs[h] = sh
                return cl

            def make_proj(qb, shs, blk):
                def cl():
                    pr = pacc.tile([128, D], f32, tag="qk", name="pr")
                    nc.tensor.matmul(pr, shs[0][:, blk * 128:(blk + 1) * 128],
                                     wo0_s, start=True, stop=False)
                    nc.tensor.matmul(pr, shs[1][:, blk * 128:(blk + 1) * 128],
                                     wo1_s, start=False, stop=True)
                    rows = slice(qb * QB + blk * 128,
                                 qb * QB + (blk + 1) * 128)
                    o_t = outp.tile([128, D], f32, tag="o", name="o_t")
                    nc.vector.tensor_copy(o_t, pr)
                    nc.gpsimd.dma_start(out=out[rows, :], in_=o_t)
                return cl

            # ---- the flat pipeline ------------------------------------
            emit_group(0)
            emit_group(1)

            schedule = defaultdict(list)
            for T in range(NT + LAG):
                if T < NT:
                    if T in (4, 5):
                        emit_group_part(2, T - 4)
                    elif T in (8, 9):
                        emit_group_part(3, T - 8)
                    if T in (2, 10, 18, 26, 34, 42):
                        issue_mask_batch(16 + (T - 2), 8)
                    emit_st(T)
                for cl in schedule.pop(T, []):
                    cl()
                if T >= LAG:
                    TT = T - LAG
                    emit_av(TT)
                    qb2, kt2 = divmod(TT, NKT)
                    if kt2 == NKT - 1 and qb2 < NQB - 1:
                        st8 = stage1(qb2)
                        shs = {}
                        schedule[T + 2].append(make_sh(st8, 0, shs))
                        schedule[T + 3].append(make_sh(st8, 1, shs))
                        for b in range(4):
                            schedule[T + 5 + 2 * b].append(
                                make_proj(qb2, shs, b))

            # ---- tail: last qb, latency-optimized ---------------------
            # Unscaled per-head projections start as soon as the numerators
            # are copied out (keeps the PE warm); 1/den is transposed into
            # per-partition layout with 8 rank-1 PE matmuls (no DRAM bounce)
            # and applied as a per-partition scale, split across ACT + DVE.
            qb = NQB - 1
            n0, n1 = num.pop(qb)
            dd = small.tile([1, 2 * QB], f32, tag="dd", name="dd")
            nc.vector.tensor_copy(dd[:, 0:QB], n0[HD:HD + 1, :])
            nc.vector.tensor_copy(dd[:, QB:2 * QB], n1[HD:HD + 1, :])
            rec = small.tile([1, 2 * QB], f32, tag="rec2", name="rec")
            nc.vector.reciprocal_approx_fast(rec, dd)
            nss = []
            for h, n in ((0, n0), (1, n1)):
                ns = nsb.tile([HD, QB], bf16, tag=f"nst{h}", name=f"nst{h}")
                nc.vector.tensor_copy(ns, n[0:HD, :])
                nss.append(ns)
            prs = []
            for b in (0, 1):
                st_blk = pst.tile([128, 2 * QB], f32, tag="st", name="st_pr")
                prs.append((st_blk[:, 0:QB], st_blk[:, QB:2 * QB]))
            q0 = pacc.tile([128, D], f32, tag="qk", name="prq0")
            q1 = pacc.tile([128, D], f32, tag="qk", name="prq1")
            prs.append((q0, q1))

            def proj_mm(b):
                for h, ns, wo_s in ((0, nss[0], wo0_s), (1, nss[1], wo1_s)):
                    nc.tensor.matmul(prs[b][h],
                                     ns[:, b * 128:(b + 1) * 128],
                                     wo_s, start=True, stop=True)

            def scale_blk(b):
                t0 = small.tile([128, D], f32, tag="t0", name="t0")
                nc.scalar.mul(t0, prs[b][0], rec_t[:, b:b + 1])
                o_t = outp.tile([128, D], f32, tag="o", name="o_t")
                nc.vector.scalar_tensor_tensor(
                    o_t, prs[b][1], rec_t[:, 4 + b:4 + b + 1], t0,
                    mybir.AluOpType.mult, mybir.AluOpType.add)
                rows = slice(qb * QB + b * 128, qb * QB + (b + 1) * 128)
                nc.gpsimd.dma_start(out=out[rows, :], in_=o_t)

            proj_mm(0)
            proj_mm(1)
            proj_mm(2)
            rec_tp = pacc.tile([128, 8], f32, tag="num", name="rec_tp")
            for c in range(8):
                nc.tensor.matmul(rec_tp[:, c:c + 1],
                                 rec[:, c * 128:(c + 1) * 128], one_f,
                                 start=True, stop=True)
            rec_t = small.tile([128, 8], f32, tag="rect", name="rec_t")
            nc.vector.tensor_copy(rec_t, rec_tp)
            scale_blk(0)
            st_blk = pst.tile([128, 2 * QB], f32, tag="st", name="st_pr")
            prs.append((st_blk[:, 0:QB], st_blk[:, QB:2 * QB]))
            proj_mm(3)
            scale_blk(1)
            scale_blk(2)
            scale_blk(3)

    nc.compile()
    return nc


def _get_compiled():
    global _compiled
    if _compiled is None:
        _compiled = _build()
    return _compiled


def kernel(gene_emb, expr_emb, V, M, fused_W, fused_b, Wq, bq, Wk, bk,
           out_W, out_b):
    gene_emb = np.asarray(gene_emb, dtype=np.float32)
    expr_emb = np.asarray(expr_emb, dtype=np.float32)
    V = np.asarray(V, dtype=np.float32)
    M = np.asarray(M, dtype=np.float32)
    fused_W = np.asarray(fused_W, dtype=np.float32)
    fused_b = np.asarray(fused_b, dtype=np.float32)
    Wq_ = np.asarray(Wq, dtype=np.float32)
    bq_ = np.asarray(bq, dtype=np.float32)
    Wk_ = np.asarray(Wk, dtype=np.float32)
    bk_ = np.asarray(bk, dtype=np.float32)
    out_W = np.asarray(out_W, dtype=np.float32)
    out_b = np.asarray(out_b, dtype=np.float32)

    nc = _get_compiled()

    import ml_dtypes
    m_np = ml_dtypes.bfloat16
    p_np = ml_dtypes.bfloat16
    f8_np = ml_dtypes.float8_e4m3

    def to_f8(a):
        return np.clip(a, -240.0, 240.0).astype(f8_np)

    fW = fused_W.astype(np.float64)
    Wqc = (fW @ Wq_.astype(np.float64)) * SCALE * QSH
    bqc = fused_b.astype(np.float64) @ Wq_.astype(np.float64) * SCALE + bq_ * SCALE
    Wkc = (fW @ Wk_.astype(np.float64)) * KSH
    bkc = fused_b.astype(np.float64) @ Wk_.astype(np.float64) + bk_

    def chunk_major(a, nch):  # [nch*128, F] -> [128, nch, F]
        F = a.shape[1]
        return np.ascontiguousarray(
            a.reshape(nch, 128, F).transpose(1, 0, 2))

    xt_b, mt_b = [], []
    for b in range(B):
        XT = np.concatenate([gene_emb[b], expr_emb[b]], axis=-1).T  # [1024,S]
        xg = XT.reshape(NCH, 128, NG, QB).transpose(1, 2, 0, 3)
        xt_b.append(to_f8(np.ascontiguousarray(
            xg.reshape(128, NG * NCH, QB))))
        # mask layout [128 keys-part, (qb,kt)=T * 512 queries]
        mtt = M[b].T.reshape(NKT, KT, NQB, QB).transpose(1, 2, 0, 3)
        mt_b.append(np.ascontiguousarray(
            mtt.reshape(KT, NT * QB)).astype(m_np))

    # V padded to 128 cols: ones at col 64 (denominator), zeros above
    # (NumWeights==128 enables fast-weight-load on the AV matmuls).
    vpad = np.zeros((S, VP - HD - 1), np.float32)
    ones_col = np.ones((S, 1), np.float32)
    in_maps = []
    for d in range(N_CORES):
        b, p = d // 4, d % 4
        h0 = 2 * p
        cols = slice(p * 128, (p + 1) * 128)
        vs = []
        for h in (h0, h0 + 1):
            Vh = np.concatenate([V[b, :, h, :], ones_col, vpad], axis=1)
            vs.append(chunk_major(Vh, NKT).astype(p_np))
        in_maps.append({
            "xt": xt_b[b],
            "mt": mt_b[b],
            "wq": to_f8(chunk_major(Wqc[:, cols].astype(np.float32), NCH)),
            "wk": to_f8(chunk_major(Wkc[:, cols].astype(np.float32), NCH)),
            "bq": bqc[cols].astype(np.float32).reshape(128, 1),
            "bk": bkc[cols].astype(np.float32).reshape(128, 1),
            "v0": vs[0],
            "v1": vs[1],
            "wo0": np.ascontiguousarray(
                out_W[h0 * HD:(h0 + 1) * HD, :]).astype(ml_dtypes.bfloat16),
            "wo1": np.ascontiguousarray(
                out_W[(h0 + 1) * HD:(h0 + 2) * HD, :]).astype(
                    ml_dtypes.bfloat16),
        })

    global _last_results
    n_run = int(os.environ.get("KERNEL_CORES", N_CORES))
    if n_run < N_CORES:
        in_maps = in_maps[:1] * N_CORES
    res = run_bass_kernel_spmd(nc, in_maps[:n_run],
                               core_ids=list(range(n_run)))
    if n_run < N_CORES:
        res.results = list(res.results) * (N_CORES // n_run)
    _last_results = res

    final = np.broadcast_to(out_b, (B, S, D)).astype(np.float32).copy()
    for d in range(N_CORES):
        final[d // 4] += res.results[d]["out"]
    return final
